# revision 1
# baseline (speedup 1.0000x reference)
"""TRN2 Bass kernel for nn_Base_1348619731207 (gnn_message_passing).

Model:
  graph_out = MLP_graph(mean_pool(x, batch))            # [B, G]
  node_out[b, n] = MLP_node_n(x[b, n, :])               # per-node MLPs, [B, N]
  out = concat([graph_out, node_out], axis=1)           # [B, G + N]

Sharding (8 cores): expert-parallel over the node dim N (64 nodes/core,
per-node head weights sliced with their nodes) + graph-parallel pooling
(16 graphs/core stream their own x rows for the mean-pool + graph head).
Each core reads ~1/8 of every tensor; no collectives; the pooling and
graph head fully overlap the DMA-bound node-head stream.

Matmuls run in fp32r (fp32 with an 11-bit mantissa; single-pass on the
PE vs 2-pass fp32; inputs pre-rounded on host). Node pairs share each
matmul's 256-wide moving operand so the big matmuls have free-dim 256.
"""

import numpy as np

import concourse.bass as bass
import concourse.mybir as mybir
from concourse import bacc
from concourse.bass_utils import run_bass_kernel_spmd
from concourse.masks import make_identity
from concourse.tile import TileContext

F32 = mybir.dt.float32
F32R = mybir.dt.float32r
RELU = mybir.ActivationFunctionType.Relu
IDENT = mybir.ActivationFunctionType.Identity

B, N, H = 128, 512, 256          # graphs, nodes/graph, hidden
DS, D1, D2, G = 128, 256, 128, 32
NCORES = 8
NPC = N // NCORES                # 64 nodes per core
NPAIR = NPC // 2                 # 32 node pairs per core
GPC = B // NCORES                # 16 graphs per core
PT = 68                          # pooling row tiles per core (68*128 = 8704 rows)

_CACHE = {}


def _round_fp32r(a: np.ndarray) -> np.ndarray:
    """Round fp32 to fp32r (8-bit exp, 11-bit mantissa; low 12 bits zero), RNE."""
    u = np.ascontiguousarray(a, dtype=np.float32).view(np.uint32)
    low = u & np.uint32(0xFFF)
    base = u & ~np.uint32(0xFFF)
    up = (low > 0x800) | ((low == 0x800) & (((u >> np.uint32(12)) & np.uint32(1)) == 1))
    return (base + np.where(up, np.uint32(0x1000), np.uint32(0))).view(np.float32)


def _build_nc():
    nc = bacc.Bacc("TRN2", target_bir_lowering=False, debug=False)

    # Per-core inputs (shapes identical on every core).
    xt_d = nc.dram_tensor("xt", [NPAIR, 128, 512], F32R, kind="ExternalInput")
    w1_d = nc.dram_tensor("w1", [NPAIR, 128, 1024], F32R, kind="ExternalInput")
    w2_d = nc.dram_tensor("w2", [NPAIR, 128, 512], F32R, kind="ExternalInput")
    w3_d = nc.dram_tensor("w3", [128, NPC], F32R, kind="ExternalInput")
    b1_d = nc.dram_tensor("b1", [128, 2 * NPC], F32, kind="ExternalInput")
    b2_d = nc.dram_tensor("b2", [128, NPC], F32, kind="ExternalInput")
    b3_d = nc.dram_tensor("b3", [128, NPC], F32, kind="ExternalInput")
    xg_d = nc.dram_tensor("xg", [PT, 128, 256], F32R, kind="ExternalInput")
    ind_d = nc.dram_tensor("ind", [128, PT * GPC], F32R, kind="ExternalInput")
    gw1_d = nc.dram_tensor("gw1", [128, 256], F32, kind="ExternalInput")
    gw2_d = nc.dram_tensor("gw2", [128, 128], F32, kind="ExternalInput")
    gw3_d = nc.dram_tensor("gw3", [128, 256], F32, kind="ExternalInput")
    gw4_d = nc.dram_tensor("gw4", [128, 256], F32, kind="ExternalInput")
    gw5_d = nc.dram_tensor("gw5", [128, G], F32, kind="ExternalInput")
    gb1_d = nc.dram_tensor("gb1", [128, 1], F32, kind="ExternalInput")
    gb2_d = nc.dram_tensor("gb2", [128, 1], F32, kind="ExternalInput")
    gb3_d = nc.dram_tensor("gb3", [128, 2], F32, kind="ExternalInput")
    gb4_d = nc.dram_tensor("gb4", [128, 1], F32, kind="ExternalInput")
    gb5_d = nc.dram_tensor("gb5", [G, 1], F32, kind="ExternalInput")

    nout_d = nc.dram_tensor("nout", [128, NPC], F32, kind="ExternalOutput")
    gout_d = nc.dram_tensor("gout", [G, GPC], F32, kind="ExternalOutput")

    with TileContext(nc) as tc:
        with (
            tc.tile_pool(name="const", bufs=1) as cst,
            tc.tile_pool(name="stream", bufs=12) as stp,
            tc.tile_pool(name="act", bufs=6) as actp,
            tc.tile_pool(name="xgp", bufs=16) as xgp,
            tc.tile_pool(name="psA", bufs=4, space=bass.MemorySpace.PSUM) as psA,
            tc.tile_pool(name="psB", bufs=3, space=bass.MemorySpace.PSUM) as psB,
            tc.tile_pool(name="psD", bufs=1, space=bass.MemorySpace.PSUM) as psD,
        ):
            # --- constants ---
            w3t = cst.tile([128, NPC], F32R)
            nc.sync.dma_start(w3t[:], w3_d[:])
            b1t = cst.tile([128, 2 * NPC], F32)
            nc.sync.dma_start(b1t[:], b1_d[:])
            b2t = cst.tile([128, NPC], F32)
            nc.sync.dma_start(b2t[:], b2_d[:])
            b3t = cst.tile([128, NPC], F32)
            nc.sync.dma_start(b3t[:], b3_d[:])
            indt = cst.tile([128, PT * GPC], F32R)
            nc.sync.dma_start(indt[:], ind_d[:])
            zeros = cst.tile([128, 128], F32)
            nc.gpsimd.memset(zeros[:], 0.0)

            def relu_bias(out, in_, bias, j):
                """relu(in_ + bias): ACT for j=0, DVE for j=1 (load balance)."""
                if j == 0:
                    nc.scalar.activation(out, in_, RELU, bias=bias)
                else:
                    nc.vector.scalar_tensor_tensor(
                        out, in_, bias, zeros[:, :in_.shape[-1]],
                        mybir.AluOpType.add, mybir.AluOpType.max,
                    )

            # node_out staging in SBUF (written column-pair at a time)
            nout_sb = cst.tile([128, NPC], F32)
            # pooling accumulator (interleaved with the node-pair loop)
            pp = psD.tile([GPC, 256], F32)

            def pool_tile(t):
                xg = xgp.tile([128, 256], F32R, tag="xg")
                nc.sync.dma_start(xg[:], xg_d[t])
                nc.tensor.matmul(
                    pp[:],
                    indt[:, t * GPC:(t + 1) * GPC],
                    xg[:],
                    start=(t == 0), stop=(t == PT - 1),
                    skip_group_check=True,
                )

            # --- node-pair loop (2 pooling tiles folded into each pair) ---
            for i in range(NPAIR):
                xt = stp.tile([128, 512], F32R, tag="xt")
                nc.sync.dma_start(xt[:], xt_d[i])
                w1 = stp.tile([128, 1024], F32R, tag="w1")
                nc.sync.dma_start(w1[:], w1_d[i])
                w2 = stp.tile([128, 512], F32R, tag="w2")
                nc.sync.dma_start(w2[:], w2_d[i])

                # consecutive matmuls alternate psum banks (j innermost);
                # each (j, mh) accumulation group closes before the same
                # bank's next group opens
                h1 = actp.tile([128, 512], F32R, tag="h1")
                p1a = psA.tile([128, 512], F32, tag="p1")
                p1b = psA.tile([128, 512], F32, tag="p1")
                p1s = [p1a, p1b]
                for mh in range(2):
                    for kh in range(2):
                        for j in range(2):
                            nc.tensor.matmul(
                                p1s[j][:, mh * 256:(mh + 1) * 256],
                                w1[:, ((j * 2 + kh) * 2 + mh) * 128:
                                     ((j * 2 + kh) * 2 + mh + 1) * 128],
                                xt[:, kh * 256:(kh + 1) * 256],
                                start=(kh == 0), stop=(kh == 1),
                            )
                for mh in range(2):
                    for j in range(2):
                        # keep only this node's 128 columns of the 256-wide out
                        relu_bias(
                            h1[:, mh * 256 + j * 128: mh * 256 + (j + 1) * 128],
                            p1s[j][:, mh * 256 + j * 128: mh * 256 + (j + 1) * 128],
                            b1t[:, (2 * i + j) * 2 + mh:(2 * i + j) * 2 + mh + 1],
                            j,
                        )

                p2a = psB.tile([128, 256], F32, tag="p2")
                p2b = psB.tile([128, 256], F32, tag="p2")
                p2s = [p2a, p2b]
                h2 = actp.tile([128, 256], F32R, tag="h2")
                for dh in range(2):
                    for j in range(2):
                        nc.tensor.matmul(
                            p2s[j][:],
                            w2[:, (j * 2 + dh) * 128:(j * 2 + dh + 1) * 128],
                            h1[:, dh * 256:(dh + 1) * 256],
                            start=(dh == 0), stop=(dh == 1),
                        )
                for j in range(2):
                    relu_bias(
                        h2[:, j * 128:(j + 1) * 128],
                        p2s[j][:, j * 128:(j + 1) * 128],
                        b2t[:, 2 * i + j:2 * i + j + 1],
                        j,
                    )

                # L3: fp32r needs even dst columns -> N=2 per node (one
                # garbage column each), staged per-j then combined with b3
                p3a = psB.tile([128, 2], F32, tag="p2")
                p3b = psB.tile([128, 2], F32, tag="p2")
                for j, p3 in ((0, p3a), (1, p3b)):
                    nc.tensor.matmul(
                        p3[:],
                        h2[:, j * 128:(j + 1) * 128],
                        w3t[:, 2 * i:2 * i + 2],
                        start=True, stop=True,
                    )
                for j, p3 in ((0, p3a), (1, p3b)):
                    n_loc = 2 * i + j
                    nc.vector.tensor_add(
                        nout_sb[:, n_loc:n_loc + 1],
                        p3[:, j:j + 1],
                        b3t[:, n_loc:n_loc + 1],
                    )

                # front-loaded: pooling finishes ~9 pairs early so the
                # graph head overlaps the remaining node-head stream
                for t in (3 * i, 3 * i + 1, 3 * i + 2):
                    if t < PT:
                        pool_tile(t)

            nc.sync.dma_start(nout_d[:], nout_sb[:])

            # --- graph head (tiny, fp32) ---
            gw1t = cst.tile([128, 256], F32)
            nc.sync.dma_start(gw1t[:], gw1_d[:])
            gw2t = cst.tile([128, 128], F32)
            nc.sync.dma_start(gw2t[:], gw2_d[:])
            gw3t = cst.tile([128, 256], F32)
            nc.sync.dma_start(gw3t[:], gw3_d[:])
            gw4t = cst.tile([128, 256], F32)
            nc.sync.dma_start(gw4t[:], gw4_d[:])
            gw5t = cst.tile([128, G], F32)
            nc.sync.dma_start(gw5t[:], gw5_d[:])
            gb1t = cst.tile([128, 1], F32)
            nc.sync.dma_start(gb1t[:], gb1_d[:])
            gb2t = cst.tile([128, 1], F32)
            nc.sync.dma_start(gb2t[:], gb2_d[:])
            gb3t = cst.tile([128, 2], F32)
            nc.sync.dma_start(gb3t[:], gb3_d[:])
            gb4t = cst.tile([128, 1], F32)
            nc.sync.dma_start(gb4t[:], gb4_d[:])
            gb5t = cst.tile([G, 1], F32)
            nc.sync.dma_start(gb5t[:], gb5_d[:])
            ident = cst.tile([128, 128], F32)
            make_identity(nc, ident[:])
            # relu(mean) then transpose [GPC, 256] -> [256, GPC]
            xgr = actp.tile([GPC, 256], F32, tag="xgr")
            nc.scalar.activation(xgr[:], pp[:], RELU)
            xgt = actp.tile([128, 2 * GPC], F32, tag="xgt")
            for kh in range(2):
                ptr = psB.tile([128, GPC], F32, tag="p2")
                nc.tensor.transpose(
                    ptr[:], xgr[:, kh * 128:(kh + 1) * 128], ident[:GPC, :GPC]
                )
                nc.vector.tensor_copy(xgt[:, kh * GPC:(kh + 1) * GPC], ptr[:])

            # layer 1: relu(x_graph) @ gs_w1 + gs_b1   (no relu after)
            g1 = psB.tile([128, GPC], F32, tag="p2")
            for kh in range(2):
                nc.tensor.matmul(
                    g1[:], gw1t[:, kh * 128:(kh + 1) * 128],
                    xgt[:, kh * GPC:(kh + 1) * GPC],
                    start=(kh == 0), stop=(kh == 1),
                )
            a1 = actp.tile([128, GPC], F32, tag="ga")
            nc.scalar.activation(a1[:], g1[:], IDENT, bias=gb1t[:, 0:1])
            # layer 2: relu(a1 @ gs_w2 + gs_b2)
            g2 = psB.tile([128, GPC], F32, tag="p2")
            nc.tensor.matmul(g2[:], gw2t[:], a1[:], start=True, stop=True)
            a2 = actp.tile([128, GPC], F32, tag="ga")
            nc.scalar.activation(a2[:], g2[:], RELU, bias=gb2t[:, 0:1])
            # layer 3: relu(a2 @ gh_w1 + gh_b1)  (D1 = 256 -> two M halves)
            a3 = actp.tile([128, 2 * GPC], F32, tag="ga3")
            for mh in range(2):
                g3 = psB.tile([128, GPC], F32, tag="p2")
                nc.tensor.matmul(
                    g3[:], gw3t[:, mh * 128:(mh + 1) * 128], a2[:],
                    start=True, stop=True,
                )
                nc.scalar.activation(
                    a3[:, mh * GPC:(mh + 1) * GPC], g3[:], RELU,
                    bias=gb3t[:, mh:mh + 1],
                )
            # layer 4: relu(a3 @ gh_w2 + gh_b2)
            g4 = psB.tile([128, GPC], F32, tag="p2")
            for kh in range(2):
                nc.tensor.matmul(
                    g4[:], gw4t[:, kh * 128:(kh + 1) * 128],
                    a3[:, kh * GPC:(kh + 1) * GPC],
                    start=(kh == 0), stop=(kh == 1),
                )
            a4 = actp.tile([128, GPC], F32, tag="ga")
            nc.scalar.activation(a4[:], g4[:], RELU, bias=gb4t[:, 0:1])
            # layer 5: a4 @ gh_w3 + gh_b3
            g5 = psB.tile([G, GPC], F32, tag="p2")
            nc.tensor.matmul(g5[:], gw5t[:], a4[:], start=True, stop=True)
            gout_sb = actp.tile([G, GPC], F32, tag="gout")
            nc.scalar.activation(gout_sb[:], g5[:], IDENT, bias=gb5t[:, 0:1])
            nc.sync.dma_start(gout_d[:], gout_sb[:])

    nc.compile()
    return nc


def _prep_core_inputs(c, xr, batch, lo_hi, inv_counts, nh_w1, nh_b1, nh_w2,
                      nh_b2, nh_w3, nh_b3, gh_consts):
    ns = slice(c * NPC, (c + 1) * NPC)
    xv = xr.reshape(B, N, H)

    # xt: [pair, p(h%128), kh, j, b] -> [NPAIR, 128, 512]
    xt = (
        xv[:, ns, :]                              # [b, n, h]
        .reshape(B, NPAIR, 2, 2, 128)             # b, pair, j, kh, p
        .transpose(1, 4, 3, 2, 0)                 # pair, p, kh, j, b
        .reshape(NPAIR, 128, 512)
    )
    xt = np.ascontiguousarray(xt)

    w1 = (
        nh_w1[ns]                                 # [n, h, d1]
        .reshape(NPAIR, 2, 2, 128, 2, 128)        # pair, j, kh, p, mh, m
        .transpose(0, 3, 1, 2, 4, 5)              # pair, p, j, kh, mh, m
        .reshape(NPAIR, 128, 1024)
    )
    w1 = _round_fp32r(w1)

    w2 = (
        nh_w2[ns]                                 # [n, d1, d2]
        .reshape(NPAIR, 2, 2, 128, 128)           # pair, j, dh, p, d2
        .transpose(0, 3, 1, 2, 4)                 # pair, p, j, dh, d2
        .reshape(NPAIR, 128, 512)
    )
    w2 = _round_fp32r(w2)

    w3 = _round_fp32r(np.ascontiguousarray(nh_w3[ns, :, 0].T))      # [128, NPC]
    b1 = np.ascontiguousarray(
        nh_b1[ns].reshape(NPC, 2, 128).transpose(2, 0, 1).reshape(128, 2 * NPC)
    )
    b2 = np.ascontiguousarray(nh_b2[ns].T)                          # [128, NPC]
    b3 = np.ascontiguousarray(
        np.broadcast_to(nh_b3[ns].reshape(1, NPC), (128, NPC))
    )

    # pooling rows for graphs [GPC*c, GPC*(c+1))
    lo, hi = lo_hi[c]
    nrows = hi - lo
    xg = np.zeros((PT * 128, 256), np.float32)
    xg[:nrows] = xr[lo:hi]
    xg = xg.reshape(PT, 128, 256)
    ind = np.zeros((PT * 128, GPC), np.float32)
    gl = batch[lo:hi] - GPC * c
    ind[np.arange(nrows), gl] = inv_counts[batch[lo:hi]]
    ind = np.ascontiguousarray(
        ind.reshape(PT, 128, GPC).transpose(1, 0, 2).reshape(128, PT * GPC)
    )
    ind = _round_fp32r(ind)

    d = {
        "xt": xt, "w1": w1, "w2": w2, "w3": w3,
        "b1": b1, "b2": b2, "b3": b3,
        "xg": xg, "ind": ind,
    }
    d.update(gh_consts)
    return d


def kernel(x, batch, gs_w1, gs_b1, gs_w2, gs_b2,
           gh_w1, gh_b1, gh_w2, gh_b2, gh_w3, gh_b3,
           nh_w1, nh_b1, nh_w2, nh_b2, nh_w3, nh_b3):
    x = np.asarray(x, np.float32)
    batch = np.asarray(batch, np.int32)

    counts = np.bincount(batch, minlength=B).astype(np.float32)
    inv_counts = np.where(counts > 0, 1.0 / np.maximum(counts, 1), 0.0).astype(
        np.float32
    )
    # row ranges per core (batch is sorted); must fit in the padded tile count
    bounds = np.searchsorted(batch, np.arange(0, B + 1, GPC))
    lo_hi = [(int(bounds[c]), int(bounds[c + 1])) for c in range(NCORES)]
    assert all(hi - lo <= PT * 128 for lo, hi in lo_hi), "graph slice too large"

    xr = _round_fp32r(x)

    gh_consts = {
        "gw1": np.ascontiguousarray(
            np.asarray(gs_w1, np.float32).reshape(2, 128, 128)
            .transpose(1, 0, 2).reshape(128, 256)
        ),
        "gw2": np.ascontiguousarray(np.asarray(gs_w2, np.float32)),
        "gw3": np.ascontiguousarray(np.asarray(gh_w1, np.float32)),
        "gw4": np.ascontiguousarray(
            np.asarray(gh_w2, np.float32).reshape(2, 128, 128)
            .transpose(1, 0, 2).reshape(128, 256)
        ),
        "gw5": np.ascontiguousarray(np.asarray(gh_w3, np.float32)),
        "gb1": np.asarray(gs_b1, np.float32).reshape(128, 1),
        "gb2": np.asarray(gs_b2, np.float32).reshape(128, 1),
        "gb3": np.ascontiguousarray(
            np.asarray(gh_b1, np.float32).reshape(2, 128).T
        ),
        "gb4": np.asarray(gh_b2, np.float32).reshape(128, 1),
        "gb5": np.asarray(gh_b3, np.float32).reshape(G, 1),
    }

    nh_w1 = np.asarray(nh_w1, np.float32)
    nh_w2 = np.asarray(nh_w2, np.float32)
    nh_w3 = np.asarray(nh_w3, np.float32)
    nh_b1 = np.asarray(nh_b1, np.float32)
    nh_b2 = np.asarray(nh_b2, np.float32)
    nh_b3 = np.asarray(nh_b3, np.float32)

    if "nc" not in _CACHE:
        _CACHE["nc"] = _build_nc()
    nc = _CACHE["nc"]

    in_maps = [
        _prep_core_inputs(c, xr, batch, lo_hi, inv_counts, nh_w1, nh_b1,
                          nh_w2, nh_b2, nh_w3, nh_b3, gh_consts)
        for c in range(NCORES)
    ]

    res = run_bass_kernel_spmd(nc, in_maps, core_ids=list(range(NCORES)))
    _CACHE["last_result"] = res

    out = np.empty((B, G + N), np.float32)
    for c in range(NCORES):
        out[GPC * c:GPC * (c + 1), :G] = res.results[c]["gout"].T
        out[:, G + NPC * c:G + NPC * (c + 1)] = res.results[c]["nout"]
    return out



# revision 3
# speedup vs baseline: 1.7885x; 1.7885x over previous
"""TRN2 Bass kernel for nn_Base_1348619731207 (gnn_message_passing).

Model:
  graph_out = MLP_graph(mean_pool(x, batch))            # [B, G]
  node_out[b, n] = MLP_node_n(x[b, n, :])               # per-node MLPs, [B, N]
  out = concat([graph_out, node_out], axis=1)           # [B, G + N]

Sharding (8 cores): expert-parallel over the node dim N (64 nodes/core,
per-node head weights sliced with their nodes) + graph-parallel pooling
(16 graphs/core stream their own x rows for the mean-pool + graph head).
No collectives; the pooling + graph head overlap the node-head stream.

Memory regime: node stream (x slice + w1 + w2) is bf16 (halves HBM
traffic and PE moving cycles vs fp32r; bf16 runs 1 cycle/row even at
128-wide moving). The pooling stream (x rows + segment-mean indicator)
is fp8 e3m4; the indicator is pre-scaled by 512 so count-reciprocals sit
in e3m4's normal range, undone via the relu's scale (relu is
positively homogeneous). The per-node loop is software-pipelined
(L1(n) | L2(n-1) | L3(n-2)) so the PE never waits on ACT/DVE relu.
"""

import numpy as np
import ml_dtypes

import concourse.bass as bass
import concourse.mybir as mybir
from concourse import bacc
from concourse.bass_utils import run_bass_kernel_spmd
from concourse.masks import make_identity
from concourse.tile import TileContext

F32 = mybir.dt.float32
BF16 = mybir.dt.bfloat16
FP8 = mybir.dt.float8e3            # e3m4
RELU = mybir.ActivationFunctionType.Relu
IDENT = mybir.ActivationFunctionType.Identity

NP_BF16 = np.dtype(ml_dtypes.bfloat16)
NP_FP8 = np.dtype(ml_dtypes.float8_e3m4)

B, N, H = 128, 512, 256          # graphs, nodes/graph, hidden
DS, D1, D2, G = 128, 256, 128, 32
NCORES = 8
NPC = N // NCORES                # 64 nodes per core
GPC = B // NCORES                # 16 graphs per core
PT = 68                          # pooling row tiles per core (68*128 = 8704 rows)
XGPACK = 4                       # pooling row tiles packed per DMA
IND_SCALE = 512.0                # keeps 1/count in e3m4 normal range

_CACHE = {}


def _build_nc():
    nc = bacc.Bacc("TRN2", target_bir_lowering=False, debug=False)

    # Per-core inputs (shapes identical on every core).
    # stream: per node 1024 cols = [xt(256: kh*128+b) | w1(512: (kh,mh)*128+m)
    # | w2(256: dh*128+m)], two nodes per DMA tile.
    st_d = nc.dram_tensor("st", [NPC // 2, 128, 2048], BF16, kind="ExternalInput")
    w3_d = nc.dram_tensor("w3", [128, NPC], BF16, kind="ExternalInput")
    b1_d = nc.dram_tensor("b1", [128, 2 * NPC], F32, kind="ExternalInput")
    b2_d = nc.dram_tensor("b2", [128, NPC], F32, kind="ExternalInput")
    b3_d = nc.dram_tensor("b3", [128, NPC], F32, kind="ExternalInput")
    xg_d = nc.dram_tensor("xg", [PT // XGPACK, 128, 256 * XGPACK], FP8,
                          kind="ExternalInput")
    ind_d = nc.dram_tensor("ind", [128, PT * GPC], FP8, kind="ExternalInput")
    gw1_d = nc.dram_tensor("gw1", [128, 256], F32, kind="ExternalInput")
    gw2_d = nc.dram_tensor("gw2", [128, 128], F32, kind="ExternalInput")
    gw3_d = nc.dram_tensor("gw3", [128, 256], F32, kind="ExternalInput")
    gw4_d = nc.dram_tensor("gw4", [128, 256], F32, kind="ExternalInput")
    gw5_d = nc.dram_tensor("gw5", [128, G], F32, kind="ExternalInput")
    gb1_d = nc.dram_tensor("gb1", [128, 1], F32, kind="ExternalInput")
    gb2_d = nc.dram_tensor("gb2", [128, 1], F32, kind="ExternalInput")
    gb3_d = nc.dram_tensor("gb3", [128, 2], F32, kind="ExternalInput")
    gb4_d = nc.dram_tensor("gb4", [128, 1], F32, kind="ExternalInput")
    gb5_d = nc.dram_tensor("gb5", [G, 1], F32, kind="ExternalInput")

    nout_d = nc.dram_tensor("nout", [128, NPC], F32, kind="ExternalOutput")
    gout_d = nc.dram_tensor("gout", [G, GPC], F32, kind="ExternalOutput")

    with TileContext(nc) as tc:
        with (
            tc.tile_pool(name="const", bufs=1) as cst,
            tc.tile_pool(name="stream", bufs=8) as stp,
            tc.tile_pool(name="act", bufs=4) as actp,
            tc.tile_pool(name="xgp", bufs=4) as xgp,
            tc.tile_pool(name="psA", bufs=3, space=bass.MemorySpace.PSUM) as psA,
            tc.tile_pool(name="psB", bufs=3, space=bass.MemorySpace.PSUM) as psB,
            tc.tile_pool(name="psC", bufs=1, space=bass.MemorySpace.PSUM) as psC,
            tc.tile_pool(name="psD", bufs=1, space=bass.MemorySpace.PSUM) as psD,
        ):
            # --- constants ---
            w3t = cst.tile([128, NPC], BF16)
            nc.sync.dma_start(w3t[:], w3_d[:])
            b1t = cst.tile([128, 2 * NPC], F32)
            nc.sync.dma_start(b1t[:], b1_d[:])
            b2t = cst.tile([128, NPC], F32)
            nc.sync.dma_start(b2t[:], b2_d[:])
            b3t = cst.tile([128, NPC], F32)
            nc.sync.dma_start(b3t[:], b3_d[:])
            indt = cst.tile([128, PT * GPC], FP8)
            nc.sync.dma_start(indt[:], ind_d[:])
            gw1t = cst.tile([128, 256], F32)
            nc.sync.dma_start(gw1t[:], gw1_d[:])
            gw2t = cst.tile([128, 128], F32)
            nc.sync.dma_start(gw2t[:], gw2_d[:])
            gw3t = cst.tile([128, 256], F32)
            nc.sync.dma_start(gw3t[:], gw3_d[:])
            gw4t = cst.tile([128, 256], F32)
            nc.sync.dma_start(gw4t[:], gw4_d[:])
            gw5t = cst.tile([128, G], F32)
            nc.sync.dma_start(gw5t[:], gw5_d[:])
            gb1t = cst.tile([128, 1], F32)
            nc.sync.dma_start(gb1t[:], gb1_d[:])
            gb2t = cst.tile([128, 1], F32)
            nc.sync.dma_start(gb2t[:], gb2_d[:])
            gb3t = cst.tile([128, 2], F32)
            nc.sync.dma_start(gb3t[:], gb3_d[:])
            gb4t = cst.tile([128, 1], F32)
            nc.sync.dma_start(gb4t[:], gb4_d[:])
            gb5t = cst.tile([G, 1], F32)
            nc.sync.dma_start(gb5t[:], gb5_d[:])
            zeros = cst.tile([128, 128], F32)
            nc.gpsimd.memset(zeros[:], 0.0)
            ident = cst.tile([128, 128], F32)
            make_identity(nc, ident[:])

            # node_out staging: L3 accumulates into one psum tile [b, n_loc]
            p3 = psC.tile([128, NPC], F32)
            nout_sb = cst.tile([128, NPC], F32)
            # pooling accumulator
            pp = psD.tile([GPC, 256], F32)

            xg_tiles = [None] * (PT // XGPACK)

            def pool_tile(t):
                tt, q = divmod(t, XGPACK)
                if q == 0:
                    xg = xgp.tile([128, 256 * XGPACK], FP8, tag="xg")
                    nc.sync.dma_start(xg[:], xg_d[tt])
                    xg_tiles[tt] = xg
                nc.tensor.matmul(
                    pp[:],
                    indt[:, t * GPC:(t + 1) * GPC],
                    xg_tiles[tt][:, q * 256:(q + 1) * 256],
                    start=(t == 0), stop=(t == PT - 1),
                    skip_group_check=True,
                )

            st_tiles = [None] * (NPC // 2)
            p1s = [None] * NPC
            h1s = [None] * NPC
            p2s = [None] * NPC
            h2s = [None] * NPC

            def stage_l1(n):
                i, j = divmod(n, 2)
                if j == 0:
                    st = stp.tile([128, 2048], BF16, tag="st")
                    nc.sync.dma_start(st[:], st_d[i])
                    st_tiles[i] = st
                st = st_tiles[i]
                base = j * 1024
                p1 = psA.tile([128, 256], F32, tag="p1")
                for mh in range(2):
                    for kh in range(2):
                        nc.tensor.matmul(
                            p1[:, mh * 128:(mh + 1) * 128],
                            st[:, base + 256 + (kh * 2 + mh) * 128:
                                 base + 256 + (kh * 2 + mh + 1) * 128],
                            st[:, base + kh * 128:base + (kh + 1) * 128],
                            start=(kh == 0), stop=(kh == 1),
                        )
                h1 = actp.tile([128, 256], BF16, tag="h1")
                # split the two relu+bias chunks across ACT and DVE
                nc.scalar.activation(
                    h1[:, 0:128], p1[:, 0:128], RELU,
                    bias=b1t[:, 2 * n:2 * n + 1],
                )
                nc.vector.scalar_tensor_tensor(
                    h1[:, 128:256], p1[:, 128:256], b1t[:, 2 * n + 1:2 * n + 2],
                    zeros[:],
                    mybir.AluOpType.add, mybir.AluOpType.max,
                )
                p1s[n], h1s[n] = p1, h1

            def stage_l2(n):
                i, j = divmod(n, 2)
                st = st_tiles[i]
                base = j * 1024
                h1 = h1s[n]
                p2 = psB.tile([128, 128], F32, tag="p2")
                for dh in range(2):
                    nc.tensor.matmul(
                        p2[:],
                        st[:, base + 768 + dh * 128:base + 768 + (dh + 1) * 128],
                        h1[:, dh * 128:(dh + 1) * 128],
                        start=(dh == 0), stop=(dh == 1),
                    )
                h2 = actp.tile([128, 128], BF16, tag="h2")
                if n % 2 == 0:
                    nc.scalar.activation(
                        h2[:], p2[:], RELU, bias=b2t[:, n:n + 1],
                    )
                else:
                    nc.vector.scalar_tensor_tensor(
                        h2[:], p2[:], b2t[:, n:n + 1], zeros[:],
                        mybir.AluOpType.add, mybir.AluOpType.max,
                    )
                p2s[n], h2s[n] = p2, h2

            def stage_l3(n):
                nc.tensor.matmul(
                    p3[:, n:n + 1],
                    h2s[n],
                    w3t[:, n:n + 1],
                    start=True, stop=True,
                )
                h2s[n] = None

            def graph_head():
                # relu(mean/512 * 512) then transpose [GPC, 256] -> [256, GPC]
                xgr = actp.tile([GPC, 256], F32, tag="xgr")
                nc.scalar.activation(xgr[:], pp[:], RELU, scale=1.0 / IND_SCALE)
                xgt = actp.tile([128, 2 * GPC], F32, tag="xgt")
                for kh in range(2):
                    ptr = psB.tile([128, GPC], F32, tag="p2")
                    nc.tensor.transpose(
                        ptr[:], xgr[:, kh * 128:(kh + 1) * 128], ident[:GPC, :GPC]
                    )
                    nc.vector.tensor_copy(xgt[:, kh * GPC:(kh + 1) * GPC], ptr[:])

                # layer 1: relu(x_graph) @ gs_w1 + gs_b1   (no relu after)
                g1 = psB.tile([128, GPC], F32, tag="p2")
                for kh in range(2):
                    nc.tensor.matmul(
                        g1[:], gw1t[:, kh * 128:(kh + 1) * 128],
                        xgt[:, kh * GPC:(kh + 1) * GPC],
                        start=(kh == 0), stop=(kh == 1),
                    )
                a1 = actp.tile([128, GPC], F32, tag="ga")
                nc.scalar.activation(a1[:], g1[:], IDENT, bias=gb1t[:, 0:1])
                # layer 2: relu(a1 @ gs_w2 + gs_b2)
                g2 = psB.tile([128, GPC], F32, tag="p2")
                nc.tensor.matmul(g2[:], gw2t[:], a1[:], start=True, stop=True)
                a2 = actp.tile([128, GPC], F32, tag="ga")
                nc.scalar.activation(a2[:], g2[:], RELU, bias=gb2t[:, 0:1])
                # layer 3: relu(a2 @ gh_w1 + gh_b1)  (D1 = 256 -> two M halves)
                a3 = actp.tile([128, 2 * GPC], F32, tag="ga3")
                for mh in range(2):
                    g3 = psB.tile([128, GPC], F32, tag="p2")
                    nc.tensor.matmul(
                        g3[:], gw3t[:, mh * 128:(mh + 1) * 128], a2[:],
                        start=True, stop=True,
                    )
                    nc.scalar.activation(
                        a3[:, mh * GPC:(mh + 1) * GPC], g3[:], RELU,
                        bias=gb3t[:, mh:mh + 1],
                    )
                # layer 4: relu(a3 @ gh_w2 + gh_b2)
                g4 = psB.tile([128, GPC], F32, tag="p2")
                for kh in range(2):
                    nc.tensor.matmul(
                        g4[:], gw4t[:, kh * 128:(kh + 1) * 128],
                        a3[:, kh * GPC:(kh + 1) * GPC],
                        start=(kh == 0), stop=(kh == 1),
                    )
                a4 = actp.tile([128, GPC], F32, tag="ga")
                nc.scalar.activation(a4[:], g4[:], RELU, bias=gb4t[:, 0:1])
                # layer 5: a4 @ gh_w3 + gh_b3
                g5 = psB.tile([G, GPC], F32, tag="p2")
                nc.tensor.matmul(g5[:], gw5t[:], a4[:], start=True, stop=True)
                gout_sb = actp.tile([G, GPC], F32, tag="gout")
                nc.scalar.activation(gout_sb[:], g5[:], IDENT, bias=gb5t[:, 0:1])
                nc.sync.dma_start(gout_d[:], gout_sb[:])

            # --- software-pipelined node loop: L1(s) | L2(s-1) | L3(s-2),
            # pooling front-loaded 2 tiles/step (pp closes at s=33), graph
            # head interleaved at s=35 so its serial tail overlaps the
            # remaining node stream ---
            pool_t = 0
            for s in range(NPC + 2):
                if s == 35:
                    graph_head()
                if s < NPC:
                    stage_l1(s)
                if 1 <= s < NPC + 1:
                    stage_l2(s - 1)
                if s >= 2:
                    stage_l3(s - 2)
                while pool_t < min(PT, 2 * (s + 1)):
                    pool_tile(pool_t)
                    pool_t += 1

            nc.vector.tensor_add(nout_sb[:], p3[:], b3t[:])
            nc.sync.dma_start(nout_d[:], nout_sb[:])

    nc.compile()
    return nc


def _prep_core_inputs(c, x, batch, lo_hi, inv_counts,
                      nh_w1, nh_b1, nh_w2, nh_b2, nh_w3, nh_b3, gh_consts):
    ns = slice(c * NPC, (c + 1) * NPC)
    xv = x.reshape(B, N, H)

    # packed bf16 node stream: [n, p, 1024] -> pairs [n/2, p, 2048]
    S = np.empty((NPC, 128, 1024), np.float32)
    S[:, :, 0:256] = (
        xv[:, ns, :]                              # [b, n, h]
        .reshape(B, NPC, 2, 128)                  # b, n, kh, p
        .transpose(1, 3, 2, 0)                    # n, p, kh, b
        .reshape(NPC, 128, 256)
    )
    S[:, :, 256:768] = (
        nh_w1[ns]                                 # [n, h, d1]
        .reshape(NPC, 2, 128, 2, 128)             # n, kh, p, mh, m
        .transpose(0, 2, 1, 3, 4)                 # n, p, kh, mh, m
        .reshape(NPC, 128, 512)
    )
    S[:, :, 768:1024] = (
        nh_w2[ns]                                 # [n, d1, d2]
        .reshape(NPC, 2, 128, 128)                # n, dh, p, m
        .transpose(0, 2, 1, 3)                    # n, p, dh, m
        .reshape(NPC, 128, 256)
    )
    st = np.ascontiguousarray(
        S.astype(NP_BF16)
        .reshape(NPC // 2, 2, 128, 1024)
        .transpose(0, 2, 1, 3)
        .reshape(NPC // 2, 128, 2048)
    )

    w3 = np.ascontiguousarray(nh_w3[ns, :, 0].T).astype(NP_BF16)     # [128, NPC]
    b1 = np.ascontiguousarray(
        nh_b1[ns].reshape(NPC, 2, 128).transpose(2, 0, 1).reshape(128, 2 * NPC)
    )
    b2 = np.ascontiguousarray(nh_b2[ns].T)                           # [128, NPC]
    b3 = np.ascontiguousarray(
        np.broadcast_to(nh_b3[ns].reshape(1, NPC), (128, NPC))
    )

    # pooling rows for graphs [GPC*c, GPC*(c+1)), fp8 e3m4
    lo, hi = lo_hi[c]
    nrows = hi - lo
    xg = np.zeros((PT * 128, 256), NP_FP8)
    xg[:nrows] = x[lo:hi].astype(NP_FP8)
    xg = np.ascontiguousarray(
        xg.reshape(PT // XGPACK, XGPACK, 128, 256 * 1)
        .transpose(0, 2, 1, 3)
        .reshape(PT // XGPACK, 128, 256 * XGPACK)
    )
    ind = np.zeros((PT * 128, GPC), np.float32)
    gl = batch[lo:hi] - GPC * c
    ind[np.arange(nrows), gl] = IND_SCALE * inv_counts[batch[lo:hi]]
    ind = np.ascontiguousarray(
        ind.reshape(PT, 128, GPC).transpose(1, 0, 2).reshape(128, PT * GPC)
    ).astype(NP_FP8)

    d = {
        "st": st, "w3": w3, "b1": b1, "b2": b2, "b3": b3,
        "xg": xg, "ind": ind,
    }
    d.update(gh_consts)
    return d


def kernel(x, batch, gs_w1, gs_b1, gs_w2, gs_b2,
           gh_w1, gh_b1, gh_w2, gh_b2, gh_w3, gh_b3,
           nh_w1, nh_b1, nh_w2, nh_b2, nh_w3, nh_b3):
    x = np.asarray(x, np.float32)
    batch = np.asarray(batch, np.int32)

    counts = np.bincount(batch, minlength=B).astype(np.float32)
    inv_counts = np.where(counts > 0, 1.0 / np.maximum(counts, 1), 0.0).astype(
        np.float32
    )
    # row ranges per core (batch is sorted); must fit in the padded tile count
    bounds = np.searchsorted(batch, np.arange(0, B + 1, GPC))
    lo_hi = [(int(bounds[c]), int(bounds[c + 1])) for c in range(NCORES)]
    assert all(hi - lo <= PT * 128 for lo, hi in lo_hi), "graph slice too large"

    gh_consts = {
        "gw1": np.ascontiguousarray(
            np.asarray(gs_w1, np.float32).reshape(2, 128, 128)
            .transpose(1, 0, 2).reshape(128, 256)
        ),
        "gw2": np.ascontiguousarray(np.asarray(gs_w2, np.float32)),
        "gw3": np.ascontiguousarray(np.asarray(gh_w1, np.float32)),
        "gw4": np.ascontiguousarray(
            np.asarray(gh_w2, np.float32).reshape(2, 128, 128)
            .transpose(1, 0, 2).reshape(128, 256)
        ),
        "gw5": np.ascontiguousarray(np.asarray(gh_w3, np.float32)),
        "gb1": np.asarray(gs_b1, np.float32).reshape(128, 1),
        "gb2": np.asarray(gs_b2, np.float32).reshape(128, 1),
        "gb3": np.ascontiguousarray(
            np.asarray(gh_b1, np.float32).reshape(2, 128).T
        ),
        "gb4": np.asarray(gh_b2, np.float32).reshape(128, 1),
        "gb5": np.asarray(gh_b3, np.float32).reshape(G, 1),
    }

    nh_w1 = np.asarray(nh_w1, np.float32)
    nh_w2 = np.asarray(nh_w2, np.float32)
    nh_w3 = np.asarray(nh_w3, np.float32)
    nh_b1 = np.asarray(nh_b1, np.float32)
    nh_b2 = np.asarray(nh_b2, np.float32)
    nh_b3 = np.asarray(nh_b3, np.float32)

    if "nc" not in _CACHE:
        _CACHE["nc"] = _build_nc()
    nc = _CACHE["nc"]

    in_maps = [
        _prep_core_inputs(c, x, batch, lo_hi, inv_counts, nh_w1, nh_b1,
                          nh_w2, nh_b2, nh_w3, nh_b3, gh_consts)
        for c in range(NCORES)
    ]

    res = run_bass_kernel_spmd(nc, in_maps, core_ids=list(range(NCORES)))
    _CACHE["last_result"] = res

    out = np.empty((B, G + N), np.float32)
    for c in range(NCORES):
        out[GPC * c:GPC * (c + 1), :G] = res.results[c]["gout"].T
        out[:, G + NPC * c:G + NPC * (c + 1)] = res.results[c]["nout"]
    return out


# revision 6
# speedup vs baseline: 2.2573x; 1.2622x over previous
"""TRN2 Bass kernel for nn_Base_1348619731207 (gnn_message_passing).

Model:
  graph_out = MLP_graph(mean_pool(x, batch))            # [B, G]
  node_out[b, n] = MLP_node_n(x[b, n, :])               # per-node MLPs, [B, N]
  out = concat([graph_out, node_out], axis=1)           # [B, G + N]

Sharding (8 cores): expert-parallel over the node dim N (64 nodes/core,
per-node head weights sliced with their nodes) + graph-parallel pooling
(16 graphs/core stream their own x rows for the mean-pool + graph head).
No collectives.

Memory regime: node stream (x slice + w1 + w2) is bf16 (halves HBM
traffic and PE moving cycles vs fp32r; bf16 runs 1 cycle/row even at
128-wide moving). The pooling stream (x rows + segment-mean indicator)
is fp8 e3m4; the indicator is pre-scaled by 512 so count-reciprocals sit
in e3m4's normal range, undone via the relu's scale (relu is positively
homogeneous). Graph-head weights are bf16.

Schedule: per-node loop is software-pipelined (L1(s) | L2(s-2) | L3(s-4))
so the PE never waits on relu; relus are spread over ACT/DVE/GpSimd.
DMAs ride two HWDGE queues (sync + scalar) so descriptor generation on
one queue overlaps transfers of the other; constants are packed into
three DMAs so the first stream tile lands within ~3 us.
"""

import numpy as np
import ml_dtypes

import concourse.bass as bass
import concourse.mybir as mybir
from concourse import bacc
from concourse.bass_utils import run_bass_kernel_spmd
from concourse.masks import make_identity
from concourse.tile import TileContext

F32 = mybir.dt.float32
BF16 = mybir.dt.bfloat16
FP8 = mybir.dt.float8e3            # e3m4
RELU = mybir.ActivationFunctionType.Relu
IDENT = mybir.ActivationFunctionType.Identity
ADD = mybir.AluOpType.add
MAX = mybir.AluOpType.max

NP_BF16 = np.dtype(ml_dtypes.bfloat16)
NP_FP8 = np.dtype(ml_dtypes.float8_e3m4)

B, N, H = 128, 512, 256          # graphs, nodes/graph, hidden
DS, D1, D2, G = 128, 256, 128, 32
NCORES = 8
NPC = N // NCORES                # 64 nodes per core
GPC = B // NCORES                # 16 graphs per core
PT = 68                          # pooling row tiles per core (68*128 = 8704 rows)
XGPACK = 4                       # pooling row tiles packed per DMA
NXG = PT // XGPACK               # 17 pooling DMA tiles
IND_SCALE = 512.0                # keeps 1/count in e3m4 normal range

# f32 const pack columns
CB1, CB2, CB3 = 0, 128, 192
CGB1, CGB2, CGB3, CGB4, CGB5 = 256, 257, 258, 260, 261
CF32 = 262
# bf16 const pack columns
CW3, CGW1, CGW2, CGW3, CGW4, CGW5 = 0, 64, 320, 448, 704, 960
CBF = 992

_CACHE = {}


def _build_nc():
    nc = bacc.Bacc("TRN2", target_bir_lowering=False, debug=False)

    # stream: per node 1024 cols = [xt(256: kh*128+b) | w1(512: (kh,mh)*128+m)
    # | w2(256: dh*128+m)], two nodes per DMA tile.
    st_d = nc.dram_tensor("st", [NPC // 2, 128, 2048], BF16, kind="ExternalInput")
    cf_d = nc.dram_tensor("cf", [128, CF32], F32, kind="ExternalInput")
    cb_d = nc.dram_tensor("cb", [128, CBF], BF16, kind="ExternalInput")
    xg_d = nc.dram_tensor("xg", [NXG, 128, 256 * XGPACK], FP8, kind="ExternalInput")
    ind_d = nc.dram_tensor("ind", [128, PT * GPC], FP8, kind="ExternalInput")

    nout_d = nc.dram_tensor("nout", [128, NPC], F32, kind="ExternalOutput")
    gout_d = nc.dram_tensor("gout", [G, GPC], F32, kind="ExternalOutput")

    with TileContext(nc) as tc:
        with (
            tc.tile_pool(name="const", bufs=1) as cst,
            tc.tile_pool(name="stream", bufs=8) as stp,
            tc.tile_pool(name="act", bufs=4) as actp,
            tc.tile_pool(name="h2p", bufs=6) as h2p,
            tc.tile_pool(name="xgp", bufs=6) as xgp,
            tc.tile_pool(name="psA", bufs=3, space=bass.MemorySpace.PSUM) as psA,
            tc.tile_pool(name="psB", bufs=3, space=bass.MemorySpace.PSUM) as psB,
            tc.tile_pool(name="psC", bufs=1, space=bass.MemorySpace.PSUM) as psC,
            tc.tile_pool(name="psD", bufs=1, space=bass.MemorySpace.PSUM) as psD,
        ):
            # --- packed constants + first tiles, split across both queues ---
            cft = cst.tile([128, CF32], F32)
            cbt = cst.tile([128, CBF], BF16)
            indt = cst.tile([128, PT * GPC], FP8)

            st_tiles = [None] * (NPC // 2)
            xg_tiles = [None] * NXG

            def dma_stream(i):
                st = stp.tile([128, 2048], BF16, tag="st")
                (nc.sync if i % 2 == 0 else nc.scalar).dma_start(st[:], st_d[i])
                st_tiles[i] = st

            def dma_xg(tt):
                xg = xgp.tile([128, 256 * XGPACK], FP8, tag="xg")
                nc.sync.dma_start(xg[:], xg_d[tt])
                xg_tiles[tt] = xg

            nc.sync.dma_start(cft[:], cf_d[:])
            nc.scalar.dma_start(indt[:], ind_d[:])
            dma_stream(0)
            dma_stream(1)
            nc.scalar.dma_start(cbt[:], cb_d[:])
            dma_xg(0)
            dma_stream(2)
            dma_stream(3)
            dma_xg(1)

            zeros = cst.tile([128, 128], F32)
            nc.gpsimd.memset(zeros[:], 0.0)
            ident = cst.tile([128, 128], F32)
            make_identity(nc, ident[:])

            # node_out staging: L3 accumulates into one psum tile [b, n_loc]
            p3 = psC.tile([128, NPC], F32)
            nout_sb = cst.tile([128, NPC], F32)
            # pooling accumulator
            pp = psD.tile([GPC, 256], F32)

            def pool_tile(t):
                tt, q = divmod(t, XGPACK)
                nc.tensor.matmul(
                    pp[:],
                    indt[:, t * GPC:(t + 1) * GPC],
                    xg_tiles[tt][:, q * 256:(q + 1) * 256],
                    start=(t == 0), stop=(t == PT - 1),
                    skip_group_check=True,
                )

            h1s = [None] * NPC
            h2s = [None] * NPC

            def stage_l1(n):
                i, j = divmod(n, 2)
                st = st_tiles[i]
                base = j * 1024
                p1 = psA.tile([128, 256], F32, tag="p1")
                for mh in range(2):
                    for kh in range(2):
                        nc.tensor.matmul(
                            p1[:, mh * 128:(mh + 1) * 128],
                            st[:, base + 256 + (kh * 2 + mh) * 128:
                                 base + 256 + (kh * 2 + mh + 1) * 128],
                            st[:, base + kh * 128:base + (kh + 1) * 128],
                            start=(kh == 0), stop=(kh == 1),
                        )
                h1 = actp.tile([128, 256], BF16, tag="h1")
                # split the two relu+bias chunks across ACT and DVE
                nc.scalar.activation(
                    h1[:, 0:128], p1[:, 0:128], RELU,
                    bias=cft[:, CB1 + 2 * n:CB1 + 2 * n + 1],
                )
                nc.vector.scalar_tensor_tensor(
                    h1[:, 128:256], p1[:, 128:256],
                    cft[:, CB1 + 2 * n + 1:CB1 + 2 * n + 2],
                    zeros[:],
                    ADD, MAX,
                )
                h1s[n] = h1

            def stage_l2(n):
                i, j = divmod(n, 2)
                st = st_tiles[i]
                base = j * 1024
                h1 = h1s[n]
                p2 = psB.tile([128, 128], F32, tag="p2")
                for dh in range(2):
                    nc.tensor.matmul(
                        p2[:],
                        st[:, base + 768 + dh * 128:base + 768 + (dh + 1) * 128],
                        h1[:, dh * 128:(dh + 1) * 128],
                        start=(dh == 0), stop=(dh == 1),
                    )
                h2 = h2p.tile([128, 128], BF16, tag="h2")
                # L2 relu on DVE (ACT is loaded with L1 + DMA issue; GpSimd
                # cannot read PSUM)
                nc.vector.scalar_tensor_tensor(
                    h2[:], p2[:], cft[:, CB2 + n:CB2 + n + 1],
                    zeros[:], ADD, MAX,
                )
                h2s[n] = h2

            def stage_l3(n):
                nc.tensor.matmul(
                    p3[:, n:n + 1],
                    h2s[n],
                    cbt[:, CW3 + n:CW3 + n + 1],
                    start=True, stop=True,
                )
                h2s[n] = None

            def graph_head():
                # relu(scale * pp) then transpose [GPC, 256] -> [256, GPC]
                xgr = actp.tile([GPC, 256], F32, tag="xgr")
                nc.scalar.activation(xgr[:], pp[:], RELU, scale=1.0 / IND_SCALE)
                xgt = actp.tile([128, 2 * GPC], BF16, tag="xgt")
                for kh in range(2):
                    ptr = psB.tile([128, GPC], F32, tag="p2")
                    nc.tensor.transpose(
                        ptr[:], xgr[:, kh * 128:(kh + 1) * 128], ident[:GPC, :GPC]
                    )
                    nc.vector.tensor_copy(xgt[:, kh * GPC:(kh + 1) * GPC], ptr[:])

                # layer 1: relu(x_graph) @ gs_w1 + gs_b1   (no relu after)
                g1 = psB.tile([128, GPC], F32, tag="p2")
                for kh in range(2):
                    nc.tensor.matmul(
                        g1[:], cbt[:, CGW1 + kh * 128:CGW1 + (kh + 1) * 128],
                        xgt[:, kh * GPC:(kh + 1) * GPC],
                        start=(kh == 0), stop=(kh == 1),
                    )
                a1 = actp.tile([128, GPC], BF16, tag="ga")
                nc.scalar.activation(a1[:], g1[:], IDENT,
                                     bias=cft[:, CGB1:CGB1 + 1])
                # layer 2: relu(a1 @ gs_w2 + gs_b2)
                g2 = psB.tile([128, GPC], F32, tag="p2")
                nc.tensor.matmul(g2[:], cbt[:, CGW2:CGW2 + 128], a1[:],
                                 start=True, stop=True)
                a2 = actp.tile([128, GPC], BF16, tag="ga")
                nc.scalar.activation(a2[:], g2[:], RELU,
                                     bias=cft[:, CGB2:CGB2 + 1])
                # layer 3: relu(a2 @ gh_w1 + gh_b1)  (D1 = 256 -> two M halves)
                a3 = actp.tile([128, 2 * GPC], BF16, tag="ga3")
                for mh in range(2):
                    g3 = psB.tile([128, GPC], F32, tag="p2")
                    nc.tensor.matmul(
                        g3[:], cbt[:, CGW3 + mh * 128:CGW3 + (mh + 1) * 128],
                        a2[:],
                        start=True, stop=True,
                    )
                    nc.scalar.activation(
                        a3[:, mh * GPC:(mh + 1) * GPC], g3[:], RELU,
                        bias=cft[:, CGB3 + mh:CGB3 + mh + 1],
                    )
                # layer 4: relu(a3 @ gh_w2 + gh_b2)
                g4 = psB.tile([128, GPC], F32, tag="p2")
                for kh in range(2):
                    nc.tensor.matmul(
                        g4[:], cbt[:, CGW4 + kh * 128:CGW4 + (kh + 1) * 128],
                        a3[:, kh * GPC:(kh + 1) * GPC],
                        start=(kh == 0), stop=(kh == 1),
                    )
                a4 = actp.tile([128, GPC], BF16, tag="ga")
                nc.scalar.activation(a4[:], g4[:], RELU,
                                     bias=cft[:, CGB4:CGB4 + 1])
                # layer 5: a4 @ gh_w3 + gh_b3
                g5 = psB.tile([G, GPC], F32, tag="p2")
                nc.tensor.matmul(g5[:], cbt[:, CGW5:CGW5 + G], a4[:],
                                 start=True, stop=True)
                gout_sb = actp.tile([G, GPC], F32, tag="gout")
                nc.scalar.activation(gout_sb[:], g5[:], IDENT,
                                     bias=cft[:G, CGB5:CGB5 + 1])
                nc.scalar.dma_start(gout_d[:], gout_sb[:])

            # --- software-pipelined node loop: L1(s) | L2(s-2) | L3(s-4).
            # Pooling runs 2 tiles/step starting s=2 (pp closes s=35); the
            # graph head is interleaved at s=37 so its serial tail overlaps
            # the remaining node stream. Stream pairs and xg tiles are
            # prefetched 4 pairs / 2 tiles ahead on alternating queues. ---
            pool_t = 0
            for s in range(NPC + 4):
                p = 4 + s // 2
                if s % 2 == 0 and p < NPC // 2:
                    dma_stream(p)
                tt = 2 + s // 2
                if s % 2 == 1 and tt < NXG:
                    dma_xg(tt)
                if s == 37:
                    graph_head()
                if s < NPC:
                    stage_l1(s)
                if 2 <= s < NPC + 2:
                    stage_l2(s - 2)
                if s >= 4:
                    stage_l3(s - 4)
                while pool_t < min(PT, 2 * max(0, s - 1)):
                    pool_tile(pool_t)
                    pool_t += 1

            nc.vector.tensor_add(nout_sb[:], p3[:], cft[:, CB3:CB3 + 64])
            nc.sync.dma_start(nout_d[:], nout_sb[:])

    nc.compile()
    return nc


def _prep_core_inputs(c, x, batch, lo_hi, inv_counts,
                      nh_w1, nh_w2, nh_w3, cf_base, cb_base):
    ns = slice(c * NPC, (c + 1) * NPC)
    xv = x.reshape(B, N, H)

    # packed bf16 node stream: [n, p, 1024] -> pairs [n/2, p, 2048]
    S = np.empty((NPC, 128, 1024), np.float32)
    S[:, :, 0:256] = (
        xv[:, ns, :]                              # [b, n, h]
        .reshape(B, NPC, 2, 128)                  # b, n, kh, p
        .transpose(1, 3, 2, 0)                    # n, p, kh, b
        .reshape(NPC, 128, 256)
    )
    S[:, :, 256:768] = (
        nh_w1[ns]                                 # [n, h, d1]
        .reshape(NPC, 2, 128, 2, 128)             # n, kh, p, mh, m
        .transpose(0, 2, 1, 3, 4)                 # n, p, kh, mh, m
        .reshape(NPC, 128, 512)
    )
    S[:, :, 768:1024] = (
        nh_w2[ns]                                 # [n, d1, d2]
        .reshape(NPC, 2, 128, 128)                # n, dh, p, m
        .transpose(0, 2, 1, 3)                    # n, p, dh, m
        .reshape(NPC, 128, 256)
    )
    st = np.ascontiguousarray(
        S.astype(NP_BF16)
        .reshape(NPC // 2, 2, 128, 1024)
        .transpose(0, 2, 1, 3)
        .reshape(NPC // 2, 128, 2048)
    )

    cb = cb_base.copy()
    cb[:, CW3:CW3 + NPC] = nh_w3[ns, :, 0].T.astype(NP_BF16)

    # pooling rows for graphs [GPC*c, GPC*(c+1)), fp8 e3m4
    lo, hi = lo_hi[c]
    nrows = hi - lo
    xg = np.zeros((PT * 128, 256), NP_FP8)
    xg[:nrows] = x[lo:hi].astype(NP_FP8)
    xg = np.ascontiguousarray(
        xg.reshape(NXG, XGPACK, 128, 256)
        .transpose(0, 2, 1, 3)
        .reshape(NXG, 128, 256 * XGPACK)
    )
    ind = np.zeros((PT * 128, GPC), np.float32)
    gl = batch[lo:hi] - GPC * c
    ind[np.arange(nrows), gl] = IND_SCALE * inv_counts[batch[lo:hi]]
    ind = np.ascontiguousarray(
        ind.reshape(PT, 128, GPC).transpose(1, 0, 2).reshape(128, PT * GPC)
    ).astype(NP_FP8)

    return {"st": st, "cf": cf_base, "cb": cb, "xg": xg, "ind": ind}


def kernel(x, batch, gs_w1, gs_b1, gs_w2, gs_b2,
           gh_w1, gh_b1, gh_w2, gh_b2, gh_w3, gh_b3,
           nh_w1, nh_b1, nh_w2, nh_b2, nh_w3, nh_b3):
    x = np.asarray(x, np.float32)
    batch = np.asarray(batch, np.int32)

    counts = np.bincount(batch, minlength=B).astype(np.float32)
    inv_counts = np.where(counts > 0, 1.0 / np.maximum(counts, 1), 0.0).astype(
        np.float32
    )
    # row ranges per core (batch is sorted); must fit in the padded tile count
    bounds = np.searchsorted(batch, np.arange(0, B + 1, GPC))
    lo_hi = [(int(bounds[c]), int(bounds[c + 1])) for c in range(NCORES)]
    assert all(hi - lo <= PT * 128 for lo, hi in lo_hi), "graph slice too large"

    nh_w1 = np.asarray(nh_w1, np.float32)
    nh_w2 = np.asarray(nh_w2, np.float32)
    nh_w3 = np.asarray(nh_w3, np.float32)
    nh_b1 = np.asarray(nh_b1, np.float32)
    nh_b2 = np.asarray(nh_b2, np.float32)
    nh_b3 = np.asarray(nh_b3, np.float32)

    # bf16 const pack (graph head weights; w3 slot filled per-core)
    cb_base = np.zeros((128, CBF), NP_BF16)
    cb_base[:, CGW1:CGW1 + 256] = (
        np.asarray(gs_w1, np.float32).reshape(2, 128, 128)
        .transpose(1, 0, 2).reshape(128, 256).astype(NP_BF16)
    )
    cb_base[:, CGW2:CGW2 + 128] = np.asarray(gs_w2, np.float32).astype(NP_BF16)
    cb_base[:, CGW3:CGW3 + 256] = np.asarray(gh_w1, np.float32).astype(NP_BF16)
    cb_base[:, CGW4:CGW4 + 256] = (
        np.asarray(gh_w2, np.float32).reshape(2, 128, 128)
        .transpose(1, 0, 2).reshape(128, 256).astype(NP_BF16)
    )
    cb_base[:, CGW5:CGW5 + G] = np.asarray(gh_w3, np.float32).astype(NP_BF16)

    # f32 const pack (per-core b1/b2/b3 slices differ)
    def cf_for_core(c):
        ns = slice(c * NPC, (c + 1) * NPC)
        cf = np.zeros((128, CF32), np.float32)
        cf[:, CB1:CB1 + 128] = (
            nh_b1[ns].reshape(NPC, 2, 128).transpose(2, 0, 1).reshape(128, 2 * NPC)
        )
        cf[:, CB2:CB2 + 64] = nh_b2[ns].T
        cf[:, CB3:CB3 + 64] = np.broadcast_to(nh_b3[ns].reshape(1, NPC), (128, NPC))
        cf[:, CGB1] = np.asarray(gs_b1, np.float32)
        cf[:, CGB2] = np.asarray(gs_b2, np.float32)
        cf[:, CGB3:CGB3 + 2] = np.asarray(gh_b1, np.float32).reshape(2, 128).T
        cf[:, CGB4] = np.asarray(gh_b2, np.float32)
        cf[:G, CGB5] = np.asarray(gh_b3, np.float32)
        return cf

    if "nc" not in _CACHE:
        _CACHE["nc"] = _build_nc()
    nc = _CACHE["nc"]

    in_maps = [
        _prep_core_inputs(c, x, batch, lo_hi, inv_counts,
                          nh_w1, nh_w2, nh_w3, cf_for_core(c), cb_base)
        for c in range(NCORES)
    ]

    res = run_bass_kernel_spmd(nc, in_maps, core_ids=list(range(NCORES)))
    _CACHE["last_result"] = res

    out = np.empty((B, G + N), np.float32)
    for c in range(NCORES):
        out[GPC * c:GPC * (c + 1), :G] = res.results[c]["gout"].T
        out[:, G + NPC * c:G + NPC * (c + 1)] = res.results[c]["nout"]
    return out
